# revision 31
# baseline (speedup 1.0000x reference)
"""DeepseekV3 decoder layer (MLA + SwiGLU MLP), T=2048 prefill, fp32 I/O.

Sharding v4: tensor-parallel with on-device collectives, pipelined in
token halves to hide collective latency.

- Token ownership: core c owns tiles {c, 8+c} (128 rows each), so a
  ReduceScatter over an 8-tile half delivers exactly one tile per core.
- Latent projections are contraction-sharded: each core holds a 256-row
  H-slice of q_a_w/kv_a_w + the matching x^T rows; partial projections
  for ALL tokens are AllReduced (kv, then q in two halves).
- Attention is tensor-parallel over heads (2/core, all query tokens);
  o_proj partials are ReduceScattered per half; post-norm hn^T is
  AllGathered per half; both overlap the other half's compute.
- MLP is tensor-parallel over the intermediate dim (1368/core, ragged
  last tile); down partials ReduceScattered per half.
- LN weights / softmax scale folded into weights on host; q_b and
  down_w are fp8(e4m3, scaled 2^9/2^8), everything else bf16.
"""

import numpy as np
import ml_dtypes

bfloat16 = ml_dtypes.bfloat16
f8np = ml_dtypes.float8_e4m3

T = 2048
H = 2048
NH = 16
QLR = 1536
KVLR = 512
DN = 128
DR = 64
DV = 128
INTER = 10944
NCORES = 8
TS = T // NCORES           # 256 tokens per core (2 tiles)
NST = TS // 128            # 2 tiles per core
NTT = T // 128             # 16 token tiles
NHT = NTT // 2             # 8 tiles per half
NFC = H // 128             # 16 hidden tiles
NRC = QLR // 128           # 12 q-latent tiles
NKV = KVLR // 128          # 4 kv-latent tiles
HPC = NH // NCORES         # 2 heads per core
IPC = INTER // NCORES      # 1368 intermediate per core
NIT = 11                   # local inter tiles (10 full + 1 ragged of 88)
IW = [128] * 10 + [IPC - 1280]
EPS = 1e-6
SCALE = (DN + DR) ** -0.5
THETA = 10000.0
QH = DN + DR               # 192
QB8 = 9                    # q_b fp8 scale = 2^QB8
DW8 = 8                    # down fp8 scale = 2^DW8

_CACHE = {}


def _build_module():
    import os
    MAXPH = int(os.environ.get("KERNEL_MAXPH", "9"))
    import concourse.bass as bass
    import concourse.tile as tile
    from concourse import bacc, mybir

    f32 = mybir.dt.float32
    bf16 = mybir.dt.bfloat16
    f8 = mybir.dt.float8e4
    AF = mybir.ActivationFunctionType
    ALU = mybir.AluOpType
    GRP = [list(range(NCORES))]

    nc = bacc.Bacc("TRN2", target_bir_lowering=False, debug=False,
                   enable_asserts=False, num_devices=NCORES)

    def inp(name, shape, dt):
        return nc.dram_tensor(name, list(shape), dt, kind="ExternalInput").ap()

    # per-core inputs
    x_strip = inp("x_strip", [NST, 128, H], bf16)
    xTc = inp("xTc", [NST, 128, T], bf16)
    qa_blk = inp("qa_blk", [NST, 128, QLR], bf16)
    kva_blk = inp("kva_blk", [NST, 128, KVLR + DR], bf16)
    qb_blk = inp("qb_blk", [NRC, 128, HPC * QH], f8)
    wuk = inp("wuk", [HPC, 128, NKV, 128], bf16)
    wuv = inp("wuv", [HPC, 128, NKV, DV], bf16)
    ow_blk = inp("ow_blk", [HPC, 128, H], bf16)
    gu_blk = inp("gu_blk", [2, NFC, 128, IPC], bf16)
    dw_blk = inp("dw_blk", [IPC, H], f8)
    # replicated inputs
    cosq = inp("cosq", [NTT, 128, DR // 2], bf16)
    sinq = inp("sinq", [NTT, 128, DR // 2], bf16)
    trimask = inp("trimask", [128, 128], bf16)
    eye = inp("eye", [128, 128], bf16)
    ones = inp("ones", [128, 1], bf16)

    out_strip = nc.dram_tensor("out_strip", [NST, 128, H], f32,
                               kind="ExternalOutput").ap()

    from contextlib import ExitStack
    with tile.TileContext(nc) as tc, ExitStack() as ctx:
        persist = ctx.enter_context(tc.tile_pool(name="persist", bufs=1))
        dram = ctx.enter_context(
            tc.tile_pool(name="dram", bufs=1, space="DRAM"))

        def pt(shape, dt, tag):
            return persist.tile(list(shape), dt, tag=tag, name=tag)

        eps_sb = pt([128, 1], f32, "eps")
        nc.vector.memset(eps_sb[:], EPS)
        eye_sb = pt([128, 128], bf16, "eye")
        nc.sync.dma_start(out=eye_sb[:], in_=eye[:])
        ones_sb = pt([128, 1], bf16, "ones")
        nc.sync.dma_start(out=ones_sb[:], in_=ones[:])
        tri_sb = pt([128, 128], bf16, "tri")
        nc.sync.dma_start(out=tri_sb[:], in_=trimask[:])
        zero4 = pt([128, 4], bf16, "zero4")
        nc.vector.memset(zero4[:], 0.0)
        x_sb = pt([128, NST, H], bf16, "x_sb")
        for st in range(NST):
            nc.sync.dma_start(out=x_sb[:, st, :], in_=x_strip[st])
        h2_sb = pt([128, NST, H], f32, "h2_sb")

        # DRAM bounce buffers for collectives
        g_sx_in = dram.tile([NST, 128, 1], f32, name="g_sx_in")
        g_sx_out = dram.tile([NCORES, NST, 128, 1], f32, name="g_sx_out",
                             addr_space="Shared")
        g_kv_in = dram.tile([NTT, 128, KVLR + DR], bf16, name="g_kv_in")
        g_kv_out = dram.tile([NTT, 128, KVLR + DR], bf16, name="g_kv_out",
                             addr_space="Shared")
        g_qc_in = [dram.tile([NHT, 128, QLR], bf16, name=f"g_qc_in{i}")
                   for i in range(2)]
        g_qc_out = [dram.tile([NHT, 128, QLR], bf16, name=f"g_qc_out{i}",
                              addr_space="Shared") for i in range(2)]
        g_at_in = [dram.tile([NHT, 128, H], bf16, name=f"g_at_in{i}")
                   for i in range(2)]
        g_at_out = [dram.tile([128, H], bf16, name=f"g_at_out{i}")
                    for i in range(2)]
        g_hnT_in = [dram.tile([NFC, 128, 128], bf16, name=f"g_hnT_in{i}")
                    for i in range(2)]
        g_hnT_out = [dram.tile([NCORES, NFC, 128, 128], bf16,
                               name=f"g_hnT_out{i}", addr_space="Shared")
                     for i in range(2)]
        g_mlp_in = [dram.tile([NHT, 128, H], bf16, name=f"g_mlp_in{i}")
                    for i in range(2)]
        g_mlp_out = [dram.tile([128, H], bf16, name=f"g_mlp_out{i}")
                     for i in range(2)]

        # ===== phase L: sharded latent projections + AllReduce =====
        with tc.tile_pool(name="pl", bufs=3) as pl, \
             tc.tile_pool(name="pls", bufs=1) as pls, \
             tc.tile_pool(name="pld", bufs=2) as pld:
            ssq_x = pls.tile([128, NST], f32, name="ssq_x")
            for st in range(NST):
                scr0 = pld.tile([128, H], bf16, tag="scr0", name="scr0")
                nc.scalar.activation(scr0[:], x_sb[:, st, :], AF.Square,
                                     accum_out=ssq_x[:, st:st + 1])
                nc.sync.dma_start(out=g_sx_in[st], in_=ssq_x[:, st:st + 1])
            nc.gpsimd.collective_compute(
                "AllGather", ALU.bypass, replica_groups=GRP,
                ins=[g_sx_in.opt()], outs=[g_sx_out.opt()])

            xTc_sb = pls.tile([128, NST, T], bf16, name="xTc_sb")
            for st in range(NST):
                nc.sync.dma_start(out=xTc_sb[:, st, :], in_=xTc[st])
            kvw = pls.tile([128, NST, KVLR + DR], bf16, name="kvw")
            qaw = pls.tile([128, NST, QLR], bf16, name="qaw")
            for st in range(NST):
                nc.sync.dma_start(out=kvw[:, st, :], in_=kva_blk[st])
                nc.sync.dma_start(out=qaw[:, st, :], in_=qa_blk[st])
            with tc.tile_pool(name="plkv", bufs=2, space="PSUM") as plkv:
                for tt in range(NTT):
                    ps = plkv.tile([128, KVLR + DR], f32, tag="kvps",
                                   name="kvps")
                    for st in range(NST):
                        nc.tensor.matmul(
                            ps[:, 0:512],
                            xTc_sb[:, st, tt * 128:(tt + 1) * 128],
                            kvw[:, st, 0:512],
                            start=(st == 0), stop=(st == NST - 1))
                        nc.tensor.matmul(
                            ps[:, 512:576],
                            xTc_sb[:, st, tt * 128:(tt + 1) * 128],
                            kvw[:, st, 512:576],
                            start=(st == 0), stop=(st == NST - 1))
                    kvo = pld.tile([128, KVLR + DR], bf16, tag="kvo",
                                   name="kvo")
                    nc.scalar.copy(kvo[:], ps[:])
                    nc.sync.dma_start(out=g_kv_in[tt], in_=kvo[:])
            nc.gpsimd.collective_compute(
                "AllReduce", ALU.add, replica_groups=GRP,
                ins=[g_kv_in.opt()], outs=[g_kv_out.opt()])
            with tc.tile_pool(name="plqa", bufs=2, space="PSUM") as plqa:
                for half in range(2):
                    for t8 in range(NHT):
                        tt = half * NHT + t8
                        ps = plqa.tile([128, QLR], f32, tag="qaps",
                                       name="qaps")
                        for st in range(NST):
                            for nn in range(QLR // 512):
                                nc.tensor.matmul(
                                    ps[:, nn * 512:(nn + 1) * 512],
                                    xTc_sb[:, st, tt * 128:(tt + 1) * 128],
                                    qaw[:, st, nn * 512:(nn + 1) * 512],
                                    start=(st == 0), stop=(st == NST - 1))
                        qco = pld.tile([128, QLR], bf16, tag="qco",
                                       name="qco")
                        nc.scalar.copy(qco[:], ps[:])
                        nc.sync.dma_start(out=g_qc_in[half][t8], in_=qco[:])
                    nc.gpsimd.collective_compute(
                        "AllReduce", ALU.add, replica_groups=GRP,
                        ins=[g_qc_in[half].opt()],
                        outs=[g_qc_out[half].opt()])

        # =================== phase A: attention (2 heads) ===================
        if MAXPH >= 1:
            with tc.tile_pool(name="pas", bufs=1) as pas, \
                 tc.tile_pool(name="pad", bufs=2) as pad:
                cosq_sb = pas.tile([128, NTT, DR // 2], bf16, name="cosq_sb")
                sinq_sb = pas.tile([128, NTT, DR // 2], bf16, name="sinq_sb")
                for tt in range(NTT):
                    nc.sync.dma_start(out=cosq_sb[:, tt, :], in_=cosq[tt])
                    nc.sync.dma_start(out=sinq_sb[:, tt, :], in_=sinq[tt])
                # norm factors: rstd_x for all tokens (tile tt = st*8+c8)
                ssq_all = pas.tile([128, NTT], f32, name="ssq_all")
                rstd_all = pas.tile([128, NTT], f32, name="rstd_all")
                for c8 in range(NCORES):
                    for st in range(NST):
                        tt = st * NCORES + c8
                        nc.sync.dma_start(out=ssq_all[:, tt:tt + 1],
                                          in_=g_sx_out[c8, st])
                nc.scalar.activation(rstd_all[:], ssq_all[:], AF.Ln,
                                     bias=eps_sb[:], scale=1.0 / H)
                nc.scalar.activation(rstd_all[:], rstd_all[:], AF.Exp,
                                     scale=-0.5)
                # kv: stats + norm + rope + transposes
                kT_lat = pas.tile([128, NKV, T], bf16, name="kT_lat")
                kT_rope = pas.tile([64, T], bf16, name="kT_rope")
                ch_full = pas.tile([128, NTT, KVLR], bf16, name="ch_full")
                s_ck = pas.tile([128, NTT], f32, name="s_ck")
                with tc.tile_pool(name="pakv", bufs=3) as pakv, \
                     tc.tile_pool(name="paktp", bufs=2, space="PSUM") as paktp:
                    for tt in range(NTT):
                        kvt = pakv.tile([128, KVLR + DR], bf16, tag="kvt",
                                        name="kvt")
                        nc.gpsimd.dma_start(out=kvt[:], in_=g_kv_out[tt])
                        scr = pad.tile([128, KVLR], bf16, tag="scrk",
                                       name="scrk", bufs=1)
                        nc.scalar.activation(scr[:], kvt[:, 0:512], AF.Square,
                                             accum_out=s_ck[:, tt:tt + 1])
                        t1c = pad.tile([128, 1], f32, tag="t1c", name="t1c")
                        nc.vector.tensor_mul(t1c[:], rstd_all[:, tt:tt + 1],
                                             rstd_all[:, tt:tt + 1])
                        nc.vector.tensor_mul(t1c[:], t1c[:],
                                             s_ck[:, tt:tt + 1])
                        nc.scalar.activation(t1c[:], t1c[:], AF.Ln,
                                             bias=eps_sb[:], scale=1.0 / KVLR)
                        nc.scalar.activation(t1c[:], t1c[:], AF.Exp,
                                             scale=-0.5)
                        nc.vector.tensor_mul(t1c[:], rstd_all[:, tt:tt + 1],
                                             t1c[:])
                        nc.vector.tensor_scalar_mul(ch_full[:, tt, :],
                                                    kvt[:, 0:512], t1c[:])
                        kr = pad.tile([128, DR], f32, tag="krr", name="krr")
                        krf = pad.tile([128, DR], bf16, tag="krf", name="krf")
                        nc.vector.tensor_scalar_mul(kr[:], kvt[:, 512:576],
                                                    rstd_all[:, tt:tt + 1])
                        x1 = kr[:, 0:DR:2]
                        x2 = kr[:, 1:DR:2]
                        ta = pad.tile([128, DR // 2], f32, tag="tak",
                                      name="tak")
                        tb = pad.tile([128, DR // 2], f32, tag="tbk",
                                      name="tbk")
                        nc.vector.tensor_mul(ta[:], x1, cosq_sb[:, tt, :])
                        nc.vector.tensor_mul(tb[:], x2, sinq_sb[:, tt, :])
                        nc.vector.tensor_sub(krf[:, 0:DR:2], ta[:], tb[:])
                        nc.vector.tensor_mul(ta[:], x2, cosq_sb[:, tt, :])
                        nc.vector.tensor_mul(tb[:], x1, sinq_sb[:, tt, :])
                        nc.vector.tensor_add(krf[:, 1:DR:2], ta[:], tb[:])
                        for rc in range(NKV):
                            tp = paktp.tile([128, 128], bf16, tag="tp",
                                            name="tp")
                            nc.tensor.transpose(
                                tp[:],
                                ch_full[:, tt, rc * 128:(rc + 1) * 128],
                                eye_sb[:])
                            nc.any.tensor_copy(
                                kT_lat[:, rc, tt * 128:(tt + 1) * 128],
                                tp[:])
                        tp = paktp.tile([128, 128], bf16, tag="tp", name="tp")
                        nc.tensor.transpose(tp[0:64, :], krf[:], eye_sb[:])
                        nc.any.tensor_copy(
                            kT_rope[:, tt * 128:(tt + 1) * 128], tp[0:64, :])
                wuk_sb = pas.tile([128, HPC, NKV, 128], bf16, name="wuk_sb")
                wuv_sb = pas.tile([128, HPC, NKV, DV], bf16, name="wuv_sb")
                for h in range(HPC):
                    nc.sync.dma_start(out=wuk_sb[:, h], in_=wuk[h])
                    nc.sync.dma_start(out=wuv_sb[:, h], in_=wuv[h])

                # q_b + rope + transposes
                qnT = pas.tile([128, HPC, T], bf16, name="qnT")
                qrT = pas.tile([64, HPC, T], bf16, name="qrT")
                with tc.tile_pool(name="paq", bufs=3) as paq, \
                     tc.tile_pool(name="paqs", bufs=1) as paqs, \
                     tc.tile_pool(name="pqb", bufs=2, space="PSUM") as pqb, \
                     tc.tile_pool(name="patp", bufs=2, space="PSUM") as patp:
                    qbw = paqs.tile([128, NRC, HPC * QH], f8, name="qbw")
                    for rc in range(NRC):
                        nc.sync.dma_start(out=qbw[:, rc, :], in_=qb_blk[rc])
                    for tt in range(NTT):
                        qct = paq.tile([128, QLR], bf16, tag="qct",
                                       name="qct")
                        nc.sync.dma_start(out=qct[:],
                                          in_=g_qc_out[tt // NHT][tt % NHT])
                        scrq = pad.tile([128, QLR], bf16, tag="scrq",
                                        name="scrq")
                        sqc = pad.tile([128, 1], f32, tag="sqc", name="sqc")
                        nc.scalar.activation(scrq[:], qct[:], AF.Square,
                                             accum_out=sqc[:])
                        nc.vector.tensor_mul(sqc[:], sqc[:],
                                             rstd_all[:, tt:tt + 1])
                        nc.vector.tensor_mul(sqc[:], sqc[:],
                                             rstd_all[:, tt:tt + 1])
                        nc.scalar.activation(sqc[:], sqc[:], AF.Ln,
                                             bias=eps_sb[:], scale=1.0 / QLR)
                        nc.scalar.activation(sqc[:], sqc[:], AF.Exp,
                                             scale=-0.5)
                        nc.vector.tensor_mul(sqc[:], rstd_all[:, tt:tt + 1],
                                             sqc[:])
                        qcn = paq.tile([128, QLR], bf16, tag="qcn",
                                       name="qcn")
                        nc.vector.tensor_scalar_mul(qcn[:], qct[:], sqc[:])
                        qcT_t = paq.tile([128, NRC, 128], bf16, tag="qcT_t",
                                         name="qcT_t")
                        for rc in range(NRC):
                            tp = patp.tile([128, 128], bf16, tag="tpq",
                                           name="tpq")
                            nc.tensor.transpose(
                                tp[:], qcn[:, rc * 128:(rc + 1) * 128],
                                eye_sb[:])
                            nc.any.tensor_copy(qcT_t[:, rc, :], tp[:])
                        q2 = pqb.tile([128, HPC * QH], f32, tag="q2",
                                      name="q2")
                        for rc in range(NRC):
                            nc.tensor.matmul(q2[:], qcT_t[:, rc, :],
                                             qbw[:, rc, :], start=(rc == 0),
                                             stop=(rc == NRC - 1))
                        qn2 = pad.tile([128, HPC * DN], bf16, tag="qn2",
                                       name="qn2")
                        qrr = pad.tile([128, HPC * DR], f32, tag="qrr",
                                       name="qrr")
                        qr2 = pad.tile([128, HPC * DR], bf16, tag="qr2",
                                       name="qr2")
                        ta = pad.tile([128, DR // 2], f32, tag="taq",
                                      name="taq")
                        tb = pad.tile([128, DR // 2], f32, tag="tbq",
                                      name="tbq")
                        for h in range(HPC):
                            nc.scalar.activation(qn2[:, h * DN:(h + 1) * DN],
                                                 q2[:, h * QH:h * QH + DN],
                                                 AF.Copy, scale=2.0 ** -QB8)
                            nc.scalar.activation(
                                qrr[:, h * DR:(h + 1) * DR],
                                q2[:, h * QH + DN:(h + 1) * QH],
                                AF.Copy, scale=2.0 ** -QB8)
                        for h in range(HPC):
                            x1 = qrr[:, h * DR + 0:(h + 1) * DR:2]
                            x2 = qrr[:, h * DR + 1:(h + 1) * DR:2]
                            nc.vector.tensor_mul(ta[:], x1, cosq_sb[:, tt, :])
                            nc.vector.tensor_mul(tb[:], x2, sinq_sb[:, tt, :])
                            nc.vector.tensor_sub(
                                qr2[:, h * DR + 0:(h + 1) * DR:2], ta[:],
                                tb[:])
                            nc.vector.tensor_mul(ta[:], x2, cosq_sb[:, tt, :])
                            nc.vector.tensor_mul(tb[:], x1, sinq_sb[:, tt, :])
                            nc.vector.tensor_add(
                                qr2[:, h * DR + 1:(h + 1) * DR:2], ta[:],
                                tb[:])
                        for h in range(HPC):
                            tp = patp.tile([128, 128], bf16, tag="tpq",
                                           name="tpq")
                            nc.tensor.transpose(
                                tp[:], qn2[:, h * DN:(h + 1) * DN], eye_sb[:])
                            nc.any.tensor_copy(
                                qnT[:, h, tt * 128:(tt + 1) * 128], tp[:])
                            tp = patp.tile([128, 128], bf16, tag="tpq",
                                           name="tpq")
                            nc.tensor.transpose(
                                tp[0:64, :], qr2[:, h * DR:(h + 1) * DR],
                                eye_sb[:])
                            nc.any.tensor_copy(
                                qrT[:, h, tt * 128:(tt + 1) * 128],
                                tp[0:64, :])

                # absorbed q latent, one token half at a time
                qT = pas.tile([128, HPC, NKV, T // 2], bf16, tag="qT",
                              name="qT", bufs=1)

                def absorb_half(half):
                    for h in range(HPC):
                        for rc in range(NKV):
                            for cc in range(2):
                                ch4 = 2 * half + cc
                                lp = pab.tile([128, 512], f32, tag="lp",
                                              name="lp")
                                nc.tensor.matmul(
                                    lp[:], wuk_sb[:, h, rc, :],
                                    qnT[:, h, ch4 * 512:(ch4 + 1) * 512],
                                    start=True, stop=True)
                                nc.scalar.copy(
                                    qT[:, h, rc, cc * 512:(cc + 1) * 512],
                                    lp[:])

                o_vT = pas.tile([128, HPC, T], bf16, name="o_vT")
                oln = [pas.tile([128, NHT, KVLR], bf16, tag=f"oln{h}",
                                name=f"oln{h}") for h in range(HPC)]
                ow_sb = pas.tile([128, HPC, H], bf16, name="ow_sb")
                for h in range(HPC):
                    nc.sync.dma_start(out=ow_sb[:, h, :], in_=ow_blk[h])
                DBG = bool(os.environ.get("KERNEL_DEBUG"))
                if DBG:
                    dbg_den = pas.tile([128, NTT], f32, name="dbg_den")

                def scores_chunk(qc4, h):
                    """scores+softmax+o_latent for 512 q (tiles 4qc4..4qc4+3),
                    head h -> oln[h][:, (4qc4..)%8, :]. Pools must be open."""
                    ol = [pol.tile([128, KVLR], f32, tag="ol", name="ol")
                          for _ in range(4)]
                    den = pden.tile([128, 512], f32, tag="den", name="den")
                    nc.tensor.matmul(den[:, 0:4], eye_sb[:], zero4[:],
                                     start=True, stop=False,
                                     skip_group_check=True)
                    for kt in range(4 * qc4 + 4):
                        q0 = max(0, (kt - 4 * qc4) * 128)
                        sp = psc.tile([128, 512], f32, tag="sp", name="sp")
                        qs = qc4 * 512 + q0
                        ql = (qc4 % 2) * 512 + q0
                        qw = 512 - q0
                        for rc in range(NKV):
                            nc.tensor.matmul(
                                sp[:, q0:512],
                                kT_lat[:, rc, kt * 128:(kt + 1) * 128],
                                qT[:, h, rc, ql:ql + qw],
                                start=(rc == 0), stop=False)
                        nc.tensor.matmul(
                            sp[:, q0:512],
                            kT_rope[:, kt * 128:(kt + 1) * 128],
                            qrT[:, h, qs:qs + qw],
                            start=False, stop=True)
                        eT = pad.tile([128, 512], bf16, tag="eT", name="eT",
                                      bufs=2)
                        nc.scalar.activation(eT[:, q0:512], sp[:, q0:512],
                                             AF.Exp)
                        if kt >= 4 * qc4:
                            nc.vector.tensor_mul(eT[:, q0:q0 + 128],
                                                 eT[:, q0:q0 + 128],
                                                 tri_sb[:])
                        for qt4 in range(max(0, kt - 4 * qc4), 4):
                            qt = 4 * qc4 + qt4
                            nc.tensor.matmul(
                                ol[qt4][:],
                                eT[:, qt4 * 128:(qt4 + 1) * 128],
                                ch_full[:, kt, :],
                                start=(kt == 0), stop=(kt == qt))
                            nc.tensor.matmul(
                                den[:, qt4:qt4 + 1],
                                eT[:, qt4 * 128:(qt4 + 1) * 128],
                                ones_sb[:],
                                start=False, stop=(kt == qt),
                                skip_group_check=True)
                    rinv = pad.tile([128, 4], f32, tag="rinv", name="rinv")
                    nc.vector.reciprocal(rinv[:], den[:, 0:4])
                    if DBG and h == 0:
                        nc.vector.tensor_copy(
                            dbg_den[:, qc4 * 4:(qc4 + 1) * 4], den[:, 0:4])
                    for qt4 in range(4):
                        nc.vector.tensor_scalar_mul(
                            oln[h][:, (4 * qc4 + qt4) % NHT, :], ol[qt4][:],
                            rinv[:, qt4:qt4 + 1])

                def o_v_half(half):
                    """oln -> o_vT for token half (chunks 2*half..2*half+1)."""
                    for h in range(HPC):
                        for cc in range(2):
                            ch4 = 2 * half + cc
                            olT = pad.tile([128, NKV, 512], bf16, tag="olT",
                                           name="olT")
                            for qt4 in range(4):
                                q8 = (4 * ch4 + qt4) % NHT
                                for rc in range(NKV):
                                    tp = pvt.tile([128, 128], bf16, tag="tp",
                                                  name="tp")
                                    nc.tensor.transpose(
                                        tp[:],
                                        oln[h][:, q8, rc * 128:(rc + 1) * 128],
                                        eye_sb[:])
                                    nc.any.tensor_copy(
                                        olT[:, rc, qt4 * 128:(qt4 + 1) * 128],
                                        tp[:])
                            ovp = pov.tile([128, 512], f32, tag="ovp",
                                           name="ovp")
                            for rc in range(NKV):
                                nc.tensor.matmul(
                                    ovp[:], wuv_sb[:, h, rc, :], olT[:, rc, :],
                                    start=(rc == 0), stop=(rc == NKV - 1))
                            nc.scalar.copy(
                                o_vT[:, h, ch4 * 512:(ch4 + 1) * 512], ovp[:])

                def o_proj_half(half):
                    for t8 in range(NHT):
                        qt = half * NHT + t8
                        at_ps = pop.tile([128, H], f32, tag="at_ps",
                                         name="at_ps")
                        for h in range(HPC):
                            for nn in range(H // 512):
                                nc.tensor.matmul(
                                    at_ps[:, nn * 512:(nn + 1) * 512],
                                    o_vT[:, h, qt * 128:(qt + 1) * 128],
                                    ow_sb[:, h, nn * 512:(nn + 1) * 512],
                                    start=(h == 0), stop=(h == HPC - 1))
                        at_bf = pad.tile([128, H], bf16, tag="at_bf",
                                         name="at_bf")
                        nc.scalar.copy(at_bf[:], at_ps[:])
                        nc.sync.dma_start(out=g_at_in[half][t8], in_=at_bf[:])
                    nc.gpsimd.collective_compute(
                        "ReduceScatter", ALU.add, replica_groups=GRP,
                        ins=[g_at_in[half].opt()],
                        outs=[g_at_out[half].opt()])

                def resid_norm(st):
                    """RS output -> h2, hn, hn^T -> AllGather (needs prtp)."""
                    at_s = pad.tile([128, H], bf16, tag="at_s", name="at_s",
                                    bufs=1)
                    nc.sync.dma_start(out=at_s[:], in_=g_at_out[st][:])
                    ssq2 = pad.tile([128, 1], f32, tag="ssq2", name="ssq2")
                    hn = pad.tile([128, H], bf16, tag="hn", name="hn",
                                  bufs=1)
                    nc.vector.tensor_add(h2_sb[:, st, :], x_sb[:, st, :],
                                         at_s[:])
                    scr2 = pad.tile([128, H], bf16, tag="scr2", name="scr2",
                                    bufs=1)
                    nc.vector.scalar_tensor_tensor(
                        scr2[:], h2_sb[:, st, :], 1.0, h2_sb[:, st, :],
                        ALU.bypass, ALU.mult, accum_out=ssq2[:])
                    nc.scalar.activation(ssq2[:], ssq2[:], AF.Ln,
                                         bias=eps_sb[:], scale=1.0 / H)
                    nc.scalar.activation(ssq2[:], ssq2[:], AF.Exp, scale=-0.5)
                    nc.vector.tensor_scalar_mul(hn[:], h2_sb[:, st, :],
                                                ssq2[:])
                    for fc in range(NFC):
                        tp = prtp.tile([128, 128], bf16, tag="tp", name="tp")
                        nc.tensor.transpose(
                            tp[:], hn[:, fc * 128:(fc + 1) * 128], eye_sb[:])
                        stage = pad.tile([128, 128], bf16, tag="stage3",
                                         name="stage3")
                        nc.any.tensor_copy(stage[:], tp[:])
                        nc.sync.dma_start(out=g_hnT_in[st][fc], in_=stage[:])
                    nc.gpsimd.collective_compute(
                        "AllGather", ALU.bypass, replica_groups=GRP,
                        ins=[g_hnT_in[st].opt()], outs=[g_hnT_out[st].opt()])

                # pipelined schedule
                with tc.tile_pool(name="pab", bufs=2, space="PSUM") as pab:
                    absorb_half(0)
                with tc.tile_pool(name="psc", bufs=2, space="PSUM") as psc, \
                     tc.tile_pool(name="pol", bufs=4, space="PSUM") as pol, \
                     tc.tile_pool(name="pden", bufs=1, space="PSUM") as pden:
                    for qc4 in range(2):
                        for h in range(HPC):
                            scores_chunk(qc4, h)
                with tc.tile_pool(name="pvt", bufs=2, space="PSUM") as pvt, \
                     tc.tile_pool(name="pov", bufs=2, space="PSUM") as pov:
                    o_v_half(0)
                with tc.tile_pool(name="pop", bufs=2, space="PSUM") as pop:
                    o_proj_half(0)
                with tc.tile_pool(name="pab", bufs=2, space="PSUM") as pab:
                    absorb_half(1)
                with tc.tile_pool(name="psc", bufs=2, space="PSUM") as psc, \
                     tc.tile_pool(name="pol", bufs=4, space="PSUM") as pol, \
                     tc.tile_pool(name="pden", bufs=1, space="PSUM") as pden:
                    for h in range(HPC):
                        scores_chunk(2, h)
                if MAXPH >= 2:
                    with tc.tile_pool(name="prtp", bufs=2,
                                      space="PSUM") as prtp:
                        resid_norm(0)
                with tc.tile_pool(name="psc", bufs=2, space="PSUM") as psc, \
                     tc.tile_pool(name="pol", bufs=4, space="PSUM") as pol, \
                     tc.tile_pool(name="pden", bufs=1, space="PSUM") as pden:
                    for h in range(HPC):
                        scores_chunk(3, h)
                with tc.tile_pool(name="pvt", bufs=2, space="PSUM") as pvt, \
                     tc.tile_pool(name="pov", bufs=2, space="PSUM") as pov:
                    o_v_half(1)
                with tc.tile_pool(name="pop", bufs=2, space="PSUM") as pop:
                    o_proj_half(1)
                if MAXPH >= 2:
                    with tc.tile_pool(name="prtp", bufs=2,
                                      space="PSUM") as prtp:
                        resid_norm(1)

                if DBG:
                    d_qnT = nc.dram_tensor("d_qnT", [128, HPC, T], bf16,
                                           kind="ExternalOutput").ap()
                    nc.sync.dma_start(out=d_qnT[:], in_=qnT[:])
                    d_qrT = nc.dram_tensor("d_qrT", [64, HPC, T], bf16,
                                           kind="ExternalOutput").ap()
                    nc.sync.dma_start(out=d_qrT[:], in_=qrT[:])
                    d_ovT = nc.dram_tensor("d_ovT", [128, HPC, T], bf16,
                                           kind="ExternalOutput").ap()
                    nc.sync.dma_start(out=d_ovT[:], in_=o_vT[:])
                    d_den = nc.dram_tensor("d_den", [128, NTT], f32,
                                           kind="ExternalOutput").ap()
                    nc.sync.dma_start(out=d_den[:], in_=dbg_den[:])

        # =================== phase M: MLP (TP inter) ===================
        if MAXPH >= 3:
            with tc.tile_pool(name="pm", bufs=2) as pm, \
                 tc.tile_pool(name="pmw", bufs=1) as pmw, \
                 tc.tile_pool(name="pmd", bufs=2) as pmd, \
                 tc.tile_pool(name="pmg", bufs=2, space="PSUM") as pmg, \
                 tc.tile_pool(name="pmu", bufs=2, space="PSUM") as pmu, \
                 tc.tile_pool(name="pmdn", bufs=1, space="PSUM") as pmdn:
                dw_sb = pmw.tile([128, NIT, H], f8, name="dw_sb")
                for it in range(NIT):
                    w = IW[it]
                    nc.sync.dma_start(out=dw_sb[0:w, it, :],
                                      in_=dw_blk[it * 128:it * 128 + w, :])
                THALF = T // 2
                actT = [pmw.tile([128, NIT, THALF], bf16, tag=f"actT{i}",
                                 name=f"actT{i}") for i in range(2)]
                hnT = [None, None]

                def gate_up(half):
                    for it in range(NIT):
                        w = IW[it]
                        gw = pm.tile([128, NFC, 128], bf16, tag="gw",
                                     name="gw")
                        uw = pm.tile([128, NFC, 128], bf16, tag="uw",
                                     name="uw")
                        for fc in range(NFC):
                            nc.sync.dma_start(
                                out=gw[:, fc, 0:w],
                                in_=gu_blk[0, fc, :, it * 128:it * 128 + w])
                            nc.sync.dma_start(
                                out=uw[:, fc, 0:w],
                                in_=gu_blk[1, fc, :, it * 128:it * 128 + w])
                        for tch in range(THALF // 512):
                            gp = pmg.tile([128, 512], f32, tag="gp",
                                          name="gp")
                            up = pmu.tile([128, 512], f32, tag="up",
                                          name="up")
                            for fc in range(NFC):
                                nc.tensor.matmul(
                                    gp[0:w, :], gw[:, fc, 0:w],
                                    hnT[half][:, fc,
                                              tch * 512:(tch + 1) * 512],
                                    start=(fc == 0), stop=(fc == NFC - 1))
                                nc.tensor.matmul(
                                    up[0:w, :], uw[:, fc, 0:w],
                                    hnT[half][:, fc,
                                              tch * 512:(tch + 1) * 512],
                                    start=(fc == 0), stop=(fc == NFC - 1))
                            gs = pmd.tile([128, 512], bf16, tag="gs",
                                          name="gs")
                            nc.scalar.activation(gs[0:w, :], gp[0:w, :],
                                                 AF.Silu)
                            nc.vector.tensor_mul(
                                actT[half][0:w, it,
                                           tch * 512:(tch + 1) * 512],
                                gs[0:w, :], up[0:w, :])

                def down_half(half):
                    for t8 in range(NHT):
                        dn_ps = pmdn.tile([128, H], f32, tag="dn", name="dn")
                        for it in range(NIT):
                            w = IW[it]
                            for nn in range(H // 512):
                                nc.tensor.matmul(
                                    dn_ps[:, nn * 512:(nn + 1) * 512],
                                    actT[half][0:w, it,
                                               t8 * 128:(t8 + 1) * 128],
                                    dw_sb[0:w, it, nn * 512:(nn + 1) * 512],
                                    start=(it == 0), stop=(it == NIT - 1))
                        dn_bf = pmd.tile([128, H], bf16, tag="dn_bf",
                                         name="dn_bf")
                        nc.scalar.activation(dn_bf[:], dn_ps[:], AF.Copy,
                                             scale=2.0 ** -DW8)
                        nc.sync.dma_start(out=g_mlp_in[half][t8],
                                          in_=dn_bf[:])
                    nc.gpsimd.collective_compute(
                        "ReduceScatter", ALU.add, replica_groups=GRP,
                        ins=[g_mlp_in[half].opt()],
                        outs=[g_mlp_out[half].opt()])

                def final_out(st):
                    mlp_s = pmd.tile([128, H], bf16, tag="mlp_s",
                                     name="mlp_s")
                    nc.sync.dma_start(out=mlp_s[:], in_=g_mlp_out[st][:])
                    fin = pmd.tile([128, H], f32, tag="fin", name="fin")
                    nc.vector.tensor_add(fin[:], h2_sb[:, st, :], mlp_s[:])
                    nc.sync.dma_start(out=out_strip[st], in_=fin[:])

                with tc.tile_pool(name="phn0", bufs=1) as phn0:
                    hnT[0] = phn0.tile([128, NFC, THALF], bf16, name="hnT0")
                    for c8 in range(NCORES):
                        for fc in range(NFC):
                            nc.gpsimd.dma_start(
                                out=hnT[0][:, fc, c8 * 128:(c8 + 1) * 128],
                                in_=g_hnT_out[0][c8, fc])
                    gate_up(0)
                with tc.tile_pool(name="phn1", bufs=1) as phn1:
                    hnT[1] = phn1.tile([128, NFC, THALF], bf16, name="hnT1")
                    for c8 in range(NCORES):
                        for fc in range(NFC):
                            nc.gpsimd.dma_start(
                                out=hnT[1][:, fc, c8 * 128:(c8 + 1) * 128],
                                in_=g_hnT_out[1][c8, fc])
                    down_half(0)
                    gate_up(1)
                    final_out(0)
                    down_half(1)
                    final_out(1)

        if MAXPH < 3:
            with tc.tile_pool(name="pex", bufs=2) as pex:
                for st in range(NST):
                    fin = pex.tile([128, H], f32, tag="finx", name="finx")
                    nc.vector.tensor_copy(fin[:], x_sb[:, st, :])
                    nc.sync.dma_start(out=out_strip[st], in_=fin[:])

        if os.environ.get("KERNEL_DEBUG"):
            def dump(name, gt, lead, width):
                dout = nc.dram_tensor(name, list(lead) + [128, width], bf16,
                                      kind="ExternalOutput").ap()
                import itertools
                with tc.tile_pool(name=f"dbg_{name}", bufs=2) as p:
                    if not lead:
                        t = p.tile([128, width], bf16, tag="t", name="t")
                        nc.sync.dma_start(out=t[:], in_=gt[:])
                        nc.sync.dma_start(out=dout[:], in_=t[:])
                        return
                    for idx in itertools.product(*[range(d) for d in lead]):
                        t = p.tile([128, width], bf16, tag="t", name="t")
                        nc.sync.dma_start(out=t[:], in_=gt[idx])
                        nc.sync.dma_start(out=dout[idx], in_=t[:])
            dump("d_qcr0", g_qc_out[0], (NHT,), QLR)
            dump("d_qcr1", g_qc_out[1], (NHT,), QLR)
            dump("d_kvr", g_kv_out, (NTT,), KVLR + DR)
            if MAXPH >= 1:
                dump("d_atin0", g_at_in[0], (NHT,), H)
                dump("d_atin1", g_at_in[1], (NHT,), H)
            if MAXPH >= 2:
                dump("d_at0", g_at_out[0], (), H)
                dump("d_at1", g_at_out[1], (), H)
            if MAXPH >= 3:
                dump("d_mlp0", g_mlp_out[0], (), H)
                dump("d_mlp1", g_mlp_out[1], (), H)
    nc.compile()
    return nc


def _host_prep(inputs):
    f32 = np.float32
    bf = bfloat16
    x = np.asarray(inputs["hidden_states"], f32)
    pos = np.asarray(inputs["positions"]).astype(f32)

    lnw_in = np.asarray(inputs["input_ln_w"], f32)
    q_a_w = np.asarray(inputs["q_a_w"], f32) * lnw_in[:, None]
    kv_a_w = np.asarray(inputs["kv_a_w"], f32) * lnw_in[:, None]
    q_b_w = (np.asarray(inputs["q_b_w"], f32)
             * np.asarray(inputs["q_a_ln_w"], f32)[:, None]) * SCALE
    kvln = np.asarray(inputs["kv_a_ln_w"], f32)
    w_uk = np.asarray(inputs["w_uk"], f32) * kvln[:, None, None]
    w_uv = np.asarray(inputs["w_uv"], f32) * kvln[:, None, None]
    o_w = np.asarray(inputs["o_w"], f32)
    pln = np.asarray(inputs["post_ln_w"], f32)
    gate_w = np.asarray(inputs["gate_w"], f32) * pln[:, None]
    up_w = np.asarray(inputs["up_w"], f32) * pln[:, None]
    down_w = np.asarray(inputs["down_w"], f32)

    inv_freq = 1.0 / (THETA ** (np.arange(0, DR, 2, dtype=f32) / DR))
    ang = pos[:, None] * inv_freq
    cos_t = np.cos(ang).astype(f32)
    sin_t = np.sin(ang).astype(f32)

    qb3 = q_b_w.reshape(QLR, NH, QH)
    wuk3 = w_uk.transpose(1, 2, 0)          # [NH, DN, KVLR]
    wuv3 = w_uv.transpose(1, 0, 2)          # [NH, KVLR, DV]
    ow3 = o_w.reshape(NH, DV, H)

    rep = {
        "cosq": np.ascontiguousarray(
            cos_t.astype(bf).reshape(NTT, 128, DR // 2)),
        "sinq": np.ascontiguousarray(
            sin_t.astype(bf).reshape(NTT, 128, DR // 2)),
        "trimask": np.ascontiguousarray(
            np.triu(np.ones((128, 128), f32)).astype(bf)),
        "eye": np.eye(128, dtype=bf),
        "ones": np.ones((128, 1), bf),
    }

    xT = np.ascontiguousarray(x.T)
    per_core = []
    for c in range(NCORES):
        hs = [c * HPC + h for h in range(HPC)]
        i0 = c * IPC
        gu = np.stack([
            gate_w[:, i0:i0 + IPC].reshape(NFC, 128, IPC),
            up_w[:, i0:i0 + IPC].reshape(NFC, 128, IPC)])
        # token tiles owned: c (st=0) and 8+c (st=1)
        rows = np.concatenate([x[c * 128:(c + 1) * 128],
                               x[(NCORES + c) * 128:(NCORES + c + 1) * 128]])
        m = dict(rep)
        m["x_strip"] = np.ascontiguousarray(
            rows.astype(bf).reshape(NST, 128, H))
        m["xTc"] = np.ascontiguousarray(
            xT[c * TS:(c + 1) * TS].astype(bf).reshape(NST, 128, T))
        m["qa_blk"] = np.ascontiguousarray(
            q_a_w[c * TS:(c + 1) * TS].astype(bf).reshape(NST, 128, QLR))
        m["kva_blk"] = np.ascontiguousarray(
            kv_a_w[c * TS:(c + 1) * TS].astype(bf)
            .reshape(NST, 128, KVLR + DR))
        m["qb_blk"] = np.ascontiguousarray(
            np.clip(qb3[:, hs].reshape(QLR, HPC * QH) * 2.0 ** QB8,
                    -240, 240)
            .reshape(NRC, 128, HPC * QH).astype(f8np))
        m["wuk"] = np.ascontiguousarray(
            wuk3[hs].reshape(HPC, 128, NKV, 128).astype(bf))
        m["wuv"] = np.ascontiguousarray(
            wuv3[hs].reshape(HPC, NKV, 128, DV).transpose(0, 2, 1, 3)
            .astype(bf))
        m["ow_blk"] = np.ascontiguousarray(ow3[hs].astype(bf))
        m["gu_blk"] = np.ascontiguousarray(gu.astype(bf))
        m["dw_blk"] = np.ascontiguousarray(
            np.clip(down_w[i0:i0 + IPC] * 2.0 ** DW8, -240, 240)
            .astype(f8np))
        per_core.append(m)
    return per_core


def kernel(**inputs):
    from concourse import bass_utils

    if "nc" not in _CACHE:
        _CACHE["nc"] = _build_module()
    nc = _CACHE["nc"]

    import os
    in_maps = _host_prep(inputs)
    trace = bool(os.environ.get("BASS_KERNEL_TRACE"))
    res = bass_utils.run_bass_kernel_spmd(nc, in_maps,
                                          core_ids=list(range(NCORES)),
                                          trace=trace)
    if trace and res.exec_time_ns is not None:
        print(f"HW exec time: {res.exec_time_ns} ns")
        _CACHE["last_result"] = res
    out = np.zeros((T, H), np.float32)
    for c in range(NCORES):
        s = res.results[c]["out_strip"]
        out[c * 128:(c + 1) * 128] = s[0]
        out[(NCORES + c) * 128:(NCORES + c + 1) * 128] = s[1]
    return out
